# revision 1
# baseline (speedup 1.0000x reference)
"""Paged-attention GQA decode kernel for Trainium2 (8 NeuronCores, SPMD).

Contract: kernel(**inputs) takes the FULL unsharded inputs of the reference
(q, k, v, k_cache, v_cache, slot_mapping, block_tables, context_lens) and
returns the FULL [NS, NH, HD] float32 output.

Strategy
--------
Work is flattened into uniform "pairs" = 256-token spans of one sequence.
All pairs across all 32 sequences are distributed evenly over the 8 cores,
so the single SPMD program (identical instructions on every core) is fed
per-core index/mask/qT data.  Host side, K and V rows are interleaved into
one [65536, 2048] table (with the reference's new-token scatter applied to
this copy -- slots are per-sequence disjoint, so semantics are identical).
Per 128-token block the device:
  1. indirect-DMA-gathers 128 interleaved [K|V] token rows (8KB each; the
     HW consumes one slot index per partition),
  2. PE-transposes K per kv-head (transpose-mode), computes
     scores^T[t, qh] = K @ qT as float32r single-pass matmuls (scale folded
     into qT on host), Exp on the scalar engine (no max subtraction --
     scores are O(1) for randn-scale inputs so fp32 exp is safe), zeroes
     padded tokens via a mask,
  3. accumulates numerator = exp^T.T @ V ([NH, NKV*HD] cross-product) and
     denominator = 1^T @ exp^T in PSUM across the pair,
  4. ships the full per-pair [NH, NKV*HD] numerator + [NH] denominator.
Host extracts the per-head diagonal blocks, sums partials per sequence and
divides.  float32r trades ~1e-4 relative error for single-pass PE matmuls
(fp32 matmuls are split into two HI/LO passes on TRN2).
"""

import math
import os

import numpy as np

from concourse import bacc, bass, mybir
import concourse.tile as tile
from concourse.bass_utils import run_bass_kernel_spmd

N_CORES = 8
TPB = 128          # tokens per compute block (= SBUF partitions)
BLOCKS_PER_PAIR = 2
PAIR_T = TPB * BLOCKS_PER_PAIR  # 256 tokens gathered per indirect DMA
SCALE = 0.08838834764831845     # 1/sqrt(128)

F32 = mybir.dt.float32
F32R = mybir.dt.float32r   # single-pass PE fp32 (reduced-precision multiply)
I32 = mybir.dt.int32

_prog_cache: dict = {}

LAST_EXEC_NS = None
LAST_RESULTS = None


def _build_program(p2c: int, nslots: int, nkv: int, hd: int, nh: int):
    """One SPMD program processing `p2c` pairs; per-core behavior is pure data."""
    row = nkv * hd                 # floats per token row in the flat cache
    g = nh // nkv                  # GQA group size
    assert hd == TPB, "head_dim must equal 128 for this layout"

    nc = bacc.Bacc("TRN2", target_bir_lowering=False, debug=False)

    kvcat = nc.dram_tensor("kvcat", [nslots, 2 * row], F32R, kind="ExternalInput")
    # qt payload: [qT per pair | ones column | 128x128 identity] all float32r
    qt = nc.dram_tensor("qt", [hd, p2c * nh + 1 + TPB], F32R, kind="ExternalInput")
    idx = nc.dram_tensor("idx", [TPB, p2c * BLOCKS_PER_PAIR], I32, kind="ExternalInput")
    msk = nc.dram_tensor("msk", [TPB, p2c * BLOCKS_PER_PAIR], F32, kind="ExternalInput")
    out = nc.dram_tensor("onum", [p2c, nh, nkv * hd], F32, kind="ExternalOutput")
    outd = nc.dram_tensor("oden", [p2c, nh], F32, kind="ExternalOutput")

    with tile.TileContext(nc) as tc:
        with (
            tc.tile_pool(name="const", bufs=1) as constp,
            tc.tile_pool(name="kv", bufs=3) as kvp,
            tc.tile_pool(name="kt", bufs=2) as ktp,
            tc.tile_pool(name="sm", bufs=3) as smp,
            tc.tile_pool(name="outp", bufs=2) as outp,
            tc.tile_pool(name="ktps", bufs=2, space="PSUM") as ktpsp,
            tc.tile_pool(name="scps", bufs=1, space="PSUM") as scpsp,
            tc.tile_pool(name="accps", bufs=2, space="PSUM") as accpsp,
            tc.tile_pool(name="denps", bufs=1, space="PSUM") as denpsp,
        ):
            qt_sb = constp.tile([hd, p2c * nh + 1 + TPB], F32R)
            nc.sync.dma_start(qt_sb[:], qt[:])
            ones_sb = qt_sb[:, p2c * nh: p2c * nh + 1]
            ident = qt_sb[:, p2c * nh + 1: p2c * nh + 1 + TPB]
            idx_sb = constp.tile([TPB, p2c * BLOCKS_PER_PAIR], I32)
            nc.sync.dma_start(idx_sb[:], idx[:])
            msk_sb = constp.tile([TPB, p2c * BLOCKS_PER_PAIR], F32)
            nc.sync.dma_start(msk_sb[:], msk[:])

            for p in range(p2c):
                # one gather per 128-token block pulls the interleaved
                # [K-row | V-row] (HW indirect DMA: one index per partition,
                # out-free-size consecutive elements per index)
                kv_tiles = []
                for jj in range(BLOCKS_PER_PAIR):
                    kv_tile = kvp.tile([TPB, 2 * row], F32R, tag=f"kv{jj}")
                    ioff = bass.IndirectOffsetOnAxis(
                        ap=idx_sb[:, p * BLOCKS_PER_PAIR + jj:
                                  p * BLOCKS_PER_PAIR + jj + 1],
                        axis=0,
                    )
                    nc.gpsimd.indirect_dma_start(
                        out=kv_tile[:], out_offset=None, in_=kvcat[:],
                        in_offset=ioff)
                    kv_tiles.append(kv_tile)

                num_ps = accpsp.tile([nh, nkv * hd], F32, tag="num")
                den_ps = denpsp.tile([1, nh], F32, tag="den")

                for jj in range(BLOCKS_PER_PAIR):
                    kv_tile = kv_tiles[jj]
                    sc_ps = scpsp.tile([TPB, nh], F32, tag="sc")
                    kts = []
                    for n in range(nkv):
                        kt_ps = ktpsp.tile([TPB, TPB], F32R, tag="ktp")
                        # transpose-mode (pure routing, exact, one instruction)
                        nc.tensor.transpose(
                            kt_ps[:],
                            kv_tile[:, n * hd:(n + 1) * hd],
                            ident,
                        )
                        # per-head SBUF staging so each score matmul waits only
                        # on its own copy, not on all eight
                        kt_n = ktp.tile([TPB, hd], F32R, tag=f"kt{n}")
                        if n % 2 == 0:
                            nc.vector.tensor_copy(kt_n[:], kt_ps[:])
                        else:
                            nc.scalar.activation(
                                kt_n[:], kt_ps[:],
                                mybir.ActivationFunctionType.Copy)
                        kts.append(kt_n)

                    for n in range(nkv):
                        # float32r: single-pass fp32 matmul (vs fp32's 2-pass)
                        nc.tensor.matmul(
                            sc_ps[:, n * g:(n + 1) * g],
                            lhsT=kts[n][:],
                            rhs=qt_sb[:, p * nh + n * g: p * nh + (n + 1) * g],
                            start=True, stop=True,
                        )

                    expT = smp.tile([TPB, nh], F32R, tag="expT")
                    nc.scalar.activation(
                        expT[:], sc_ps[:], mybir.ActivationFunctionType.Exp)
                    nc.vector.tensor_scalar_mul(
                        expT[:], expT[:],
                        msk_sb[:, p * BLOCKS_PER_PAIR + jj:
                               p * BLOCKS_PER_PAIR + jj + 1],
                    )

                    st = jj == 0
                    sp = jj == BLOCKS_PER_PAIR - 1
                    half = nkv * hd // 2
                    nc.tensor.matmul(
                        num_ps[:, :half], lhsT=expT[:],
                        rhs=kv_tile[:, row: row + half],
                        start=st, stop=sp)
                    nc.tensor.matmul(
                        num_ps[:, half:], lhsT=expT[:],
                        rhs=kv_tile[:, row + half: 2 * row],
                        start=st, stop=sp)
                    nc.tensor.matmul(
                        den_ps[:], lhsT=ones_sb,
                        rhs=expT[:],
                        start=st, stop=sp)

                # ship the full [nh, nkv*hd] numerator; the host extracts the
                # per-head diagonal blocks (PSUM reads must start 32-aligned,
                # so on-chip extraction would need 9 small DMAs instead)
                num_sb = outp.tile([nh, nkv * hd], F32, tag="numsb")
                den_sb = outp.tile([1, nh], F32, tag="densb")
                half = nkv * hd // 2
                nc.vector.tensor_copy(num_sb[:, :half], num_ps[:, :half])
                nc.scalar.activation(
                    num_sb[:, half:], num_ps[:, half:],
                    mybir.ActivationFunctionType.Copy)
                nc.vector.tensor_copy(den_sb[:], den_ps[:])
                nc.sync.dma_start(out[p], num_sb[:])
                nc.sync.dma_start(outd[p, None, :], den_sb[:])

    nc.compile()
    return nc


def _plan(context_lens: np.ndarray):
    """Flatten (seq, pair) work items and split them over cores."""
    ns = context_lens.shape[0]
    npairs = [(int(L) + PAIR_T - 1) // PAIR_T for L in context_lens]
    work = [(s, j) for s in range(ns) for j in range(npairs[s])]
    p2c = (len(work) + N_CORES - 1) // N_CORES
    work += [None] * (p2c * N_CORES - len(work))
    per_core = [work[c * p2c:(c + 1) * p2c] for c in range(N_CORES)]
    return p2c, per_core


def _prepare(q, k, v, k_cache, v_cache, slot_mapping, block_tables, context_lens):
    ns, nh, hd = q.shape
    nb, bs, nkv, _ = k_cache.shape
    nslots = nb * bs
    row = nkv * hd
    g = nh // nkv
    assert hd == TPB and TPB % bs == 0

    # Interleave K and V rows into one [nslots, 2*row] table so one indirect
    # DMA gathers both, and apply the reference's new-token scatter host-side
    # on this copy (slots are per-sequence disjoint, semantics identical).
    kv = np.empty((nslots, 2 * row), np.float32)
    kv[:, :row] = np.ascontiguousarray(k_cache, dtype=np.float32).reshape(nslots, row)
    kv[:, row:] = np.ascontiguousarray(v_cache, dtype=np.float32).reshape(nslots, row)
    sm = np.asarray(slot_mapping).astype(np.int64)
    kv[sm, :row] = np.asarray(k, dtype=np.float32).reshape(ns, row)
    kv[sm, row:] = np.asarray(v, dtype=np.float32).reshape(ns, row)

    cl = np.asarray(context_lens).astype(np.int64)
    bt = np.asarray(block_tables).astype(np.int64)
    p2c, per_core = _plan(cl)

    qts, idxs, msks = [], [], []
    for c in range(N_CORES):
        qt_c = np.zeros((hd, p2c * nh + 1 + TPB), np.float32)
        qt_c[:, p2c * nh] = 1.0                                   # ones column
        qt_c[:, p2c * nh + 1:] = np.eye(TPB, dtype=np.float32)    # identity
        idx_c = np.zeros((TPB, p2c * BLOCKS_PER_PAIR), np.int32)
        msk_c = np.zeros((TPB, p2c * BLOCKS_PER_PAIR), np.float32)
        for m, item in enumerate(per_core[c]):
            if item is None:
                continue
            s, j = item
            L = int(cl[s])
            nblk = (L + bs - 1) // bs
            qt_c[:, m * nh:(m + 1) * nh] = (np.asarray(q[s], np.float32) * SCALE).T
            t = j * PAIR_T + np.arange(PAIR_T, dtype=np.int64)
            cb = t // bs
            valid_cb = cb < nblk
            slot = np.where(valid_cb, bt[s, np.minimum(cb, nblk - 1)] * bs + t % bs, 0)
            cols = slice(m * BLOCKS_PER_PAIR, (m + 1) * BLOCKS_PER_PAIR)
            idx_c[:, cols] = slot.reshape(BLOCKS_PER_PAIR, TPB).T.astype(np.int32)
            msk_c[:, cols] = (t < L).reshape(BLOCKS_PER_PAIR, TPB).T.astype(np.float32)
        qts.append(qt_c)
        idxs.append(idx_c)
        msks.append(msk_c)

    in_maps = [
        {"kvcat": kv, "qt": qts[c], "idx": idxs[c], "msk": msks[c]}
        for c in range(N_CORES)
    ]
    meta = dict(ns=ns, nh=nh, hd=hd, nkv=nkv, g=g, p2c=p2c, per_core=per_core,
                nslots=nslots)
    return in_maps, meta


def _combine(results, meta):
    ns, nh, hd = meta["ns"], meta["nh"], meta["hd"]
    nkv, g = meta["nkv"], meta["g"]
    num = np.zeros((ns, nh, hd), np.float64)
    den = np.zeros((ns, nh), np.float64)
    qh = np.arange(nh)
    for c, items in enumerate(meta["per_core"]):
        onum = results[c]["onum"]
        oden = results[c]["oden"]
        for m, item in enumerate(items):
            if item is None:
                continue
            s, _ = item
            # extract per-head diagonal blocks of the [nh, nkv*hd] cross-product
            num[s] += onum[m].reshape(nh, nkv, hd)[qh, qh // g]
            den[s] += oden[m]
    return (num / den[:, :, None]).astype(np.float32)


def kernel(q, k, v, k_cache, v_cache, slot_mapping, block_tables, context_lens):
    global LAST_EXEC_NS, LAST_RESULTS
    in_maps, meta = _prepare(q, k, v, k_cache, v_cache, slot_mapping,
                             block_tables, context_lens)
    key = (meta["p2c"], meta["nslots"], meta["nkv"], meta["hd"], meta["nh"])
    if key not in _prog_cache:
        _prog_cache[key] = _build_program(*key)
    nc = _prog_cache[key]

    trace = bool(int(os.environ.get("KERNEL_TRACE", "0")))
    res = run_bass_kernel_spmd(nc, in_maps, list(range(N_CORES)), trace=trace)
    LAST_EXEC_NS = res.exec_time_ns
    LAST_RESULTS = res
    return _combine(res.results, meta)

